# revision 1
# baseline (speedup 1.0000x reference)
"""Correlation cost-volume kernel for Trainium2 (Bass/Tile).

Problem: in1, in2: [B=8, C=128, H=96, W=128] fp32.
Output: [B, 81, H, W] where out[b, dy*9+dx, y, x] =
    mean_c( in1[b,c,y,x] * in2_pad[b,c,y+dy,x+dx] ),
with in2 zero-padded by 4 in both spatial dims (max_displacement=4).

Strategy (data-parallel over batch, one sample per NeuronCore), fp16:
  - 4-row blocks: per block and per 32-pixel group g, ONE matmul with
    stationary = in1[:, yb:yb+4, 32g:32g+32] ([C, 4 rows, 32 px] ->
    M=128, PSUM partitions m = 32r+u at base 0, which is the only base
    the MM ISA allows) and moving = s2p[:, yb:yb+12, 32g:32g+40]
    ([C, 12, 40], N=480). The 12 dy' rows cover dy' = r+dy for all
    r in [0,4), dy in [0,9). fp16 runs 1 cycle/row at any N, so
    sharing the moving across 4 rows cuts PE work 3x vs per-row
    matmuls (480 vs 1440 cycles/row).
  - Full-width [128, 960] PSUM->SBUF copies (2 per block, Scalar /
    Vector) into wt4[m, blk, g, dy', v] (fp16).
  - 4 "unshift" DMAs per 32-row chunk remove the r-dependent dy'
    offset (partition base 32r, byte offset r*dy_stride: pure APs):
    wtf[32r+u, blk, g, dy, v] = wt4[32r+u, blk, g, r+dy, v].
  - 32 batched shear DMAs per chunk (one per residue u) extract the
    banded taps: t2f[m, blk, g, dy, dx] = wtf[m, blk, g, dy, u+dx]
    where u = m mod 32. (The baseline issued 32 tiny DMAs per ROW =
    3072 total; each dma_start costs the issuing engine ~0.6-1.3 us,
    which was 90% of its runtime.)
  - PE-transpose t2f[:, blk, g, :] ([128 m, 81 k] -> [81, 128]); the
    r-interleave (m = 32r+u) is undone for free by the drain/output
    access patterns: out[k, yb+r, 32g+u] <- tt[k, g, 32r+u].
  - Scalar/Vector drain PSUM->SBUF with the 1/C scale; one output DMA
    per chunk (contiguous 16 KiB per partition).

fp16 inputs: inputs are unit normals, C=128 products accumulate in
fp32 PSUM; measured end-to-end relative error ~5e-3 vs the 2e-2 gate.
"""

import numpy as np

import concourse.bass as bass
import concourse.mybir as mybir
from concourse import bacc
from concourse.bass_utils import run_bass_kernel_spmd
from concourse.masks import make_identity
from concourse.tile import TileContext

B = 8
C = 128
H = 96
W = 128
D = 9  # 2*max_disp + 1
K = D * D  # 81 output channels
PAD = 4
WP = W + 2 * PAD  # 136
FP32 = mybir.dt.float32
FP16 = mybir.dt.float16

N_CORES = 8
RCH = 32  # rows per shear chunk
NBLK = RCH // 4  # 4-row blocks per chunk
COPY = mybir.ActivationFunctionType.Copy


def build_bass(h: int = H):
    """Build the per-core Bass program for a [C, h, W] sample."""
    hp = h + 2 * PAD
    nch = h // RCH
    assert h % RCH == 0
    nc = bacc.Bacc(None, target_bir_lowering=False)
    # in1s is host-shuffled to block-major [C, blk, g, r, u] with
    # y = 4*blk + r, x = 32*g + u, so each matmul's stationary
    # ([C, 128] = 4 rows x 32 px of one group) is one contiguous dim
    # (the MM ISA allows only one free dim on the weights AP).
    in1s = nc.dram_tensor(
        "in1s", [C, h // 4, 4, 4, 32], FP16, kind="ExternalInput"
    )
    # in2p is host-padded: [C, h+8, W+8] with zeros in the 4-wide borders.
    in2p = nc.dram_tensor("in2p", [C, hp, WP], FP16, kind="ExternalInput")
    out = nc.dram_tensor("out", [K, h, W], FP32, kind="ExternalOutput")

    with TileContext(nc) as tc:
        with (
            tc.tile_pool(name="cst", bufs=1) as cst,
            tc.tile_pool(name="s1p", bufs=2) as s1p,
            tc.tile_pool(name="wt4p", bufs=2) as wt4p,
            tc.tile_pool(name="wtfp", bufs=1) as wtfp,
            tc.tile_pool(name="t2p", bufs=2) as t2p,
            tc.tile_pool(name="top", bufs=2) as top,
            tc.tile_pool(name="gpp", bufs=3, space="PSUM") as gpp,
            tc.tile_pool(name="ttp", bufs=2, space="PSUM") as ttp,
        ):
            s2p = cst.tile([C, hp, WP], FP16, name="s2p")
            ident = cst.tile([128, 128], FP16, name="ident")
            make_identity(nc, ident)

            # Load the padded in2 plane in row chunks so compute starts early.
            n2 = 4
            rows2 = (hp + n2 - 1) // n2
            for i in range(0, hp, rows2):
                r = min(rows2, hp - i)
                nc.sync.dma_start(s2p[:, i : i + r, :], in2p[:, i : i + r, :])

            for ch in range(nch):
                y0 = ch * RCH
                # s1c[c, blk, g, m] with m = 32r+u (one contiguous 128 dim)
                s1c = s1p.tile([C, NBLK, 4, 128], FP16, name="s1c", tag="s1c")
                b0 = ch * NBLK
                nc.sync.dma_start(
                    s1c[:, :, :, :].rearrange("c b g (r u) -> c b g r u", r=4),
                    in1s[:, b0 : b0 + NBLK, :, :, :],
                )

                # wt4[m=32r+u, blk, g, dy', v]
                wt4 = wt4p.tile([128, NBLK, 4, 12, 40], FP16, name="wt4", tag="wt4")
                for blk in range(NBLK):
                    yb = y0 + 4 * blk
                    # two 2-bank PSUM tiles per block: g in {0,1} / {2,3}
                    for half in range(2):
                        gp = gpp.tile([128, 2, 512], FP32, name="gp", tag="gp")
                        for j in range(2):
                            g = 2 * half + j
                            nc.tensor.matmul(
                                gp[:, j, 0:480].rearrange(
                                    "p (dy v) -> p dy v", dy=12
                                ),
                                s1c[:, blk, g, :],
                                s2p[:, yb : yb + 12, 32 * g : 32 * g + 40],
                                start=True,
                                stop=True,
                            )
                        # full-width windowed PSUM -> SBUF copy (fp32 -> fp16)
                        src = gp[:, :, 0:480].rearrange(
                            "p j (dy v) -> p j dy v", dy=12
                        )
                        dst = wt4[:, blk, 2 * half : 2 * half + 2, :, :]
                        if half == 0:
                            nc.scalar.activation(dst, src, COPY)
                        else:
                            nc.vector.tensor_copy(dst, src)

                # --- dy-unshift: 4 partition-block DMAs ---
                # wtf[32r+u, blk, g, dy, v] = wt4[32r+u, blk, g, r+dy, v]
                wtf = wtfp.tile([128, NBLK, 4, D, 40], FP16, name="wtf", tag="wtf")
                for r in range(4):
                    eng = (nc.sync, nc.gpsimd, nc.sync, nc.gpsimd)[r]
                    eng.dma_start(
                        wtf[32 * r : 32 * r + 32, :, :, :, :],
                        wt4[32 * r : 32 * r + 32, :, :, r : r + D, :],
                    )

                # --- batched band extraction: 32 partition-strided DMAs ---
                # For u = m mod 32: t2f[m, blk, g, dy, dx] = wtf[m, blk, g, dy, u+dx]
                t2f = t2p.tile([128, NBLK, 4, D, D], FP16, name="t2f", tag="t2f")
                for s in range(32):
                    src = wtf[s::32, :, :, :, s : s + D]
                    dst = t2f[s::32, :, :, :, :]
                    eng = (nc.sync, nc.scalar, nc.gpsimd, nc.sync)[s % 4]
                    eng.dma_start(dst, src)

                # --- PE transpose [128 m, 81 k] -> [81, 128 m], drain, store ---
                to = top.tile([K, RCH, W], FP32, name="to", tag="to")
                for blk in range(NBLK):
                    tt = ttp.tile([K, 4, 128], FP16, name="tt", tag="tt")
                    for g in range(4):
                        nc.tensor.transpose(
                            tt[:, g, :], t2f[:, blk, g, :, :], ident[:, :]
                        )
                    # drain + 1/C scale; undo the m = 32r+u interleave:
                    # to[k, 4blk+r, 32g+u] <- tt[k, g, 32r+u]
                    dst = to[:, 4 * blk : 4 * blk + 4, :].rearrange(
                        "k r (g u) -> k r g u", g=4
                    )
                    src = tt[:, :, :].rearrange("k g (r u) -> k r g u", r=4)
                    if blk % 2 == 0:
                        nc.scalar.activation(dst, src, COPY, scale=1.0 / C)
                    else:
                        nc.vector.tensor_scalar_mul(dst, src, 1.0 / C)

                # --- store: contiguous [81, RCH*W] block ---
                nc.sync.dma_start(out[:, y0 : y0 + RCH, :], to[:, :, :])

    nc.compile()
    return nc


_cached = {}


def _get_nc(h: int):
    if h not in _cached:
        _cached[h] = build_bass(h)
    return _cached[h]


def _pad_in2(in2: np.ndarray) -> np.ndarray:
    # [C, h, W] fp16 -> [C, h+8, W+8] zero-padded, contiguous fp16
    return np.pad(
        in2.astype(np.float16), ((0, 0), (PAD, PAD), (PAD, PAD)), mode="constant"
    )


def _shuffle_in1(in1: np.ndarray) -> np.ndarray:
    # [C, h, W] -> [C, h//4, 4(g), 4(r), 32(u)]: block-major stationary
    # layout with y = 4*blk + r, x = 32*g + u.
    c, h, w = in1.shape
    a = in1.astype(np.float16).reshape(c, h // 4, 4, 4, 32)  # c, blk, r, g, u
    return np.ascontiguousarray(a.transpose(0, 1, 3, 2, 4))  # c, blk, g, r, u


def kernel(**inputs: np.ndarray) -> np.ndarray:
    in1 = np.asarray(inputs["in1"], dtype=np.float32)
    in2 = np.asarray(inputs["in2"], dtype=np.float32)
    assert in1.shape == (B, C, H, W), in1.shape

    nc = _get_nc(H)
    in_maps = [
        {
            "in1s": _shuffle_in1(in1[b]),
            "in2p": np.ascontiguousarray(_pad_in2(in2[b])),
        }
        for b in range(B)
    ]
    res = run_bass_kernel_spmd(nc, in_maps, core_ids=list(range(N_CORES)))
    return np.stack([r["out"] for r in res.results], axis=0)



# revision 2
# speedup vs baseline: 3.8497x; 3.8497x over previous
"""Correlation cost-volume kernel for Trainium2 (Bass/Tile), v2.

Problem: in1, in2: [B=8, C=128, H=96, W=128] fp32.
Output: [B, 81, H, W] where out[b, dy*9+dx, y, x] =
    mean_c( in1[b,c,y,x] * in2_pad[b,c,y+dy,x+dx] ),
with in2 zero-padded by 4 in both spatial dims (max_displacement=4).

Data-parallel over batch (one sample per NeuronCore), fp16 compute.

v2 redesign vs v1: v1 spent ~750us of engine time issuing band-extraction
DMAs with 18-byte contiguous runs (Sync 84% busy on DIRECT2D).  v2
restructures the intermediate layouts so every shuffle DMA has long
contiguous runs and the whole gather is ~1.3k descriptors/chunk instead
of ~37k:

  - 8x16 pixel blocks (m = 16r+u): one matmul per block with stationary
    in1[:, 8 rows x 16 px] (M=128) and moving in2p[:, 16 dy', 24 v]
    (N=384, vs 480 for 4x32 blocks).
  - PSUM->SBUF copies write W[m | dy'16, v24, blkg32] with blkg
    innermost (blkg = 8*blk+g indexes the 32 pixel-blocks of a 32-row
    chunk).
  - dy-unshift (out row y = 8blk+r needs dy' = r+dy): 8 DMAs (one per
    r, 16 partitions each); slicing dy' in [r, r+9) with (v, blkg) full
    is ONE contiguous 13.8KB run per partition.
  - band extraction (x shift: out col x = 16g+u needs v = u+dx): 16
    DMAs (one per u, partitions u::16); for each (partition, dy) the
    9 dx values x 32 blkg are a contiguous 576B run on both sides
    (overlapping-window gather collapses: dst (dx, blkg) <-> src
    (v=u+dx, blkg)).
  - engine reorder copy t2f[m | dy,dx,blkg] -> t3[m | blkg, 81] to give
    the PE transpose a single contiguous free dim (ISA: stationary AP
    has one free dim).
  - PE-transpose per block-group: tt[k=81, m] <- t3[:, bg, :].
  - drain with 1/C scale to fp16, one contiguous output store per chunk;
    host upcasts to fp32.
"""

import numpy as np

import concourse.bass as bass
import concourse.mybir as mybir
from concourse import bacc
from concourse.bass_utils import run_bass_kernel_spmd
from concourse.masks import make_identity
from concourse.tile import TileContext

B = 8
C = 128
H = 96
W = 128
D = 9  # 2*max_disp + 1
K = D * D  # 81 output channels
PAD = 4
WP = W + 2 * PAD  # 136
FP32 = mybir.dt.float32
FP16 = mybir.dt.float16

N_CORES = 8
RCH = 32  # rows per chunk
BR = 8  # block rows
BU = 16  # block cols
NBLK = RCH // BR  # 4 row-blocks per chunk
NG = W // BU  # 8 col-groups
NBG = NBLK * NG  # 32 pixel-blocks per chunk
DYP = BR + 2 * PAD  # 16 dy' values per block
VP = BU + 2 * PAD  # 24 v values per group
NMM = DYP * VP  # 384 matmul free size
COPY = mybir.ActivationFunctionType.Copy


def build_bass(h: int = H):
    """Build the per-core Bass program for a [C, h, W] sample."""
    hp = h + 2 * PAD
    nch = h // RCH
    assert h % RCH == 0
    nc = bacc.Bacc(None, target_bir_lowering=False)
    # in1s is host-shuffled to [C, blk_total, g, m] with m = 16r+u,
    # y = 8*blk + r, x = 16*g + u (stationary needs one free dim).
    in1s = nc.dram_tensor("in1s", [C, h // BR, NG, 128], FP16, kind="ExternalInput")
    # in2p is host-padded: [C, h+8, W+8] with zeros in the 4-wide borders.
    in2p = nc.dram_tensor("in2p", [C, hp, WP], FP16, kind="ExternalInput")
    out = nc.dram_tensor("out", [K, h, W], FP16, kind="ExternalOutput")

    with TileContext(nc) as tc:
        with (
            tc.tile_pool(name="cst", bufs=1) as cst,
            tc.tile_pool(name="s1p", bufs=2) as s1p,
            tc.tile_pool(name="wp", bufs=2) as wp,
            tc.tile_pool(name="wtfp", bufs=2) as wtfp,
            tc.tile_pool(name="t2p", bufs=2) as t2p,
            tc.tile_pool(name="t3p", bufs=2) as t3p,
            tc.tile_pool(name="top", bufs=2) as top,
            tc.tile_pool(name="gpp", bufs=2, space="PSUM") as gpp,
            tc.tile_pool(name="ttp", bufs=2, space="PSUM") as ttp,
        ):
            s2p = cst.tile([C, hp, WP], FP16, name="s2p")
            ident = cst.tile([128, 128], FP16, name="ident")
            make_identity(nc, ident)

            # Load the padded in2 plane in row chunks so compute starts early.
            n2 = 4
            rows2 = (hp + n2 - 1) // n2
            for i in range(0, hp, rows2):
                r = min(rows2, hp - i)
                nc.gpsimd.dma_start(s2p[:, i : i + r, :], in2p[:, i : i + r, :])

            for ch in range(nch):
                y0 = ch * RCH
                # stationary for this chunk: [C, blk, g, m]
                s1c = s1p.tile([C, NBLK, NG, 128], FP16, name="s1c", tag="s1c")
                b0 = ch * NBLK
                nc.gpsimd.dma_start(s1c[:, :, :, :], in1s[:, b0 : b0 + NBLK, :, :])

                # W[m | dy', v, blkg] fp16, blkg = 8*blk + g innermost
                wt = wp.tile([128, DYP, VP, NBG], FP16, name="wt", tag="wt")
                for blk in range(NBLK):
                    yb = y0 + BR * blk  # top row of block, in padded coords
                    for half in range(NG // 2):
                        gp = gpp.tile([128, 2, 512], FP32, name="gp", tag="gp")
                        for j in range(2):
                            g = 2 * half + j
                            nc.tensor.matmul(
                                gp[:, j, 0:NMM].rearrange(
                                    "p (dy v) -> p dy v", dy=DYP
                                ),
                                s1c[:, blk, g, :],
                                s2p[:, yb : yb + DYP, BU * g : BU * g + VP],
                                start=True,
                                stop=True,
                            )
                        # PSUM -> W (fp32 -> fp16), strided dst (blkg inner)
                        bg = NG * blk + 2 * half
                        dst = wt[:, :, :, bg : bg + 2]
                        src = gp[:, :, 0:NMM].rearrange(
                            "p j (dy v) -> p dy v j", dy=DYP
                        )
                        if (blk * 4 + half) % 2 == 0:
                            nc.scalar.activation(dst, src, COPY)
                        else:
                            nc.vector.tensor_copy(dst, src)

                # --- dy-unshift: 8 DMAs, one 13.8KB contiguous run/partition
                # wtf[16r+u | dy, v, blkg] = wt[16r+u | r+dy, v, blkg]
                wtf = wtfp.tile([128, D, VP, NBG], FP16, name="wtf", tag="wtf")
                for r in range(BR):
                    eng = (nc.sync, nc.scalar)[r % 2]
                    eng.dma_start(
                        wtf[BU * r : BU * r + BU, :, :, :],
                        wt[BU * r : BU * r + BU, r : r + D, :, :],
                    )

                # --- band extraction: 16 DMAs, 576B runs both sides
                # t2f[m | dy, dx, blkg] = wtf[m | dy, u+dx, blkg], u = m%16
                t2f = t2p.tile([128, D, D, NBG], FP16, name="t2f", tag="t2f")
                for u in range(BU):
                    eng = (nc.sync, nc.scalar)[u % 2]
                    eng.dma_start(
                        t2f[u::BU, :, :, :],
                        wtf[u::BU, :, u : u + D, :],
                    )

                # --- reorder for PE transpose: t3[m | blkg, dy, dx]
                t3 = t3p.tile([128, NBG, D, D], FP16, name="t3", tag="t3")
                nc.scalar.activation(
                    t3[:, 0 : NBG // 2, :, :],
                    t2f[:, :, :, 0 : NBG // 2].rearrange("p dy dx b -> p b dy dx"),
                    COPY,
                )
                nc.vector.tensor_copy(
                    t3[:, NBG // 2 :, :, :],
                    t2f[:, :, :, NBG // 2 :].rearrange("p dy dx b -> p b dy dx"),
                )

                # --- PE transpose + drain (1/C scale, fp16) + store ---
                to = top.tile([K, RCH, W], FP16, name="to", tag="to")
                for blk in range(NBLK):
                    tt = ttp.tile([K, NG, 128], FP16, name="tt", tag="tt")
                    for g in range(NG):
                        bg = NG * blk + g
                        nc.tensor.transpose(
                            tt[:, g, :],
                            t3[:, bg, :, :].rearrange("p dy dx -> p (dy dx)"),
                            ident[:, :],
                        )
                    # to[k, 8blk+r, 16g+u] <- tt[k, g, 16r+u]
                    dst = to[:, BR * blk : BR * blk + BR, :].rearrange(
                        "k r (g u) -> k r g u", g=NG
                    )
                    src = tt[:, :, :].rearrange("k g (r u) -> k r g u", r=BR)
                    if blk % 2 == 0:
                        nc.scalar.activation(dst, src, COPY, scale=1.0 / C)
                    else:
                        nc.vector.tensor_scalar_mul(dst, src, 1.0 / C)

                # --- store: contiguous [81, RCH*W] fp16 block ---
                nc.sync.dma_start(out[:, y0 : y0 + RCH, :], to[:, :, :])

    nc.compile()
    return nc


_cached = {}


def _get_nc(h: int):
    if h not in _cached:
        _cached[h] = build_bass(h)
    return _cached[h]


def _pad_in2(in2: np.ndarray) -> np.ndarray:
    # [C, h, W] fp16 -> [C, h+8, W+8] zero-padded, contiguous fp16
    return np.pad(
        in2.astype(np.float16), ((0, 0), (PAD, PAD), (PAD, PAD)), mode="constant"
    )


def _shuffle_in1(in1: np.ndarray) -> np.ndarray:
    # [C, h, W] -> [C, h//8, 8(g), 128(m)] with m = 16r+u,
    # y = 8*blk + r, x = 16*g + u.
    c, h, w = in1.shape
    a = in1.astype(np.float16).reshape(c, h // BR, BR, NG, BU)  # c,blk,r,g,u
    a = a.transpose(0, 1, 3, 2, 4)  # c, blk, g, r, u
    return np.ascontiguousarray(a.reshape(c, h // BR, NG, 128))


def kernel(**inputs: np.ndarray) -> np.ndarray:
    in1 = np.asarray(inputs["in1"], dtype=np.float32)
    in2 = np.asarray(inputs["in2"], dtype=np.float32)
    assert in1.shape == (B, C, H, W), in1.shape

    nc = _get_nc(H)
    in_maps = [
        {
            "in1s": _shuffle_in1(in1[b]),
            "in2p": np.ascontiguousarray(_pad_in2(in2[b])),
        }
        for b in range(B)
    ]
    res = run_bass_kernel_spmd(nc, in_maps, core_ids=list(range(N_CORES)))
    return np.stack([r["out"] for r in res.results], axis=0).astype(np.float32)


# revision 4
# speedup vs baseline: 4.6194x; 1.1999x over previous
"""Correlation cost-volume kernel for Trainium2 (Bass/Tile), v3.

Problem: in1, in2: [B=8, C=128, H=96, W=128] fp32.
Output: [B, 81, H, W] where out[b, dy*9+dx, y, x] =
    mean_c( in1[b,c,y,x] * in2_pad[b,c,y+dy,x+dx] ),
with in2 zero-padded by 4 in both spatial dims (max_displacement=4).

Data-parallel over batch (one sample per NeuronCore), fp16 compute.

Pipeline per 32-row chunk (4 row-blocks x 8 col-groups of 8x16 pixels,
m = 16r+u):
  1. matmul per block: stationary in1[:, 8rows x 16px] (M=128), moving
     in2p[:, 16 dy', 24 v] (N=384) -> psum[m, dy', v].
  2. ACT/DVE drain psum -> W[m | dy'16, v24, blkg32] fp16 (blkg = 8blk+g
     innermost so the extraction DMAs below get 576B contiguous runs).
  3. u-extract (x-shift): 16 DMAs on Sync (one per u, partitions u::16):
     t2x[m | dy', dx, blkg] = W[m | dy', u+dx, blkg]; src/dst runs 576B+.
  4. r-unshift (y-shift): 8 DMAs on GpSimd (one per r, partitions
     16r..16r+16): t2f[m | dy, dx, blkg] = t2x[m | r+dy, dx, blkg]; one
     contiguous 5.2KB run per partition.
  5. DVE reorder t2f -> t3[m | blkg, 81] so the PE transpose stationary
     has a single contiguous free dim.
  6. PE transpose per block-group: tt[k=81, m] <- t3[:, bg, :].
  7. ACT/DVE drain with 1/C scale to fp16 staging, one contiguous store
     per chunk; host upcasts to fp32.

v3 vs v2: u-first extraction (1.18+0.66 MB/chunk vs 1.77+0.66), Scalar
does no DMAs (was starving the psum drains), all stationary loads are
prefetched upfront, in2p load slices match per-chunk consumption so
chunk-0 matmuls start ~6us in, PSUM pools 3x2-bank mm tiles + 2x1-bank
transpose tiles.
"""

import numpy as np

import concourse.bass as bass
import concourse.mybir as mybir
from concourse import bacc
from concourse.bass_utils import run_bass_kernel_spmd
from concourse.masks import make_identity
from concourse.tile import TileContext

B = 8
C = 128
H = 96
W = 128
D = 9  # 2*max_disp + 1
K = D * D  # 81 output channels
PAD = 4
WP = W + 2 * PAD  # 136
FP32 = mybir.dt.float32
FP16 = mybir.dt.float16

N_CORES = 8
RCH = 32  # rows per chunk
BR = 8  # block rows
BU = 16  # block cols
NBLK = RCH // BR  # 4 row-blocks per chunk
NG = W // BU  # 8 col-groups
NBG = NBLK * NG  # 32 pixel-blocks per chunk
DYP = BR + 2 * PAD  # 16 dy' values per block
VP = BU + 2 * PAD  # 24 v values per group
NMM = DYP * VP  # 384 matmul free size
COPY = mybir.ActivationFunctionType.Copy


def build_bass(h: int = H):
    """Build the per-core Bass program for a [C, h, W] sample."""
    hp = h + 2 * PAD
    nch = h // RCH
    assert h % RCH == 0
    nc = bacc.Bacc(None, target_bir_lowering=False)
    # in1s is host-shuffled to [C, blk_total, g, m] with m = 16r+u,
    # y = 8*blk + r, x = 16*g + u (stationary needs one free dim).
    in1s = nc.dram_tensor("in1s", [C, h // BR, NG, 128], FP16, kind="ExternalInput")
    # in2p is host-padded: [C, h+8, W+8] with zeros in the 4-wide borders.
    in2p = nc.dram_tensor("in2p", [C, hp, WP], FP16, kind="ExternalInput")
    out = nc.dram_tensor("out", [K, h, W], FP16, kind="ExternalOutput")

    with TileContext(nc) as tc:
        with (
            tc.tile_pool(name="cst", bufs=1) as cst,
            tc.tile_pool(name="s1p", bufs=3) as s1p,
            tc.tile_pool(name="wp", bufs=2) as wp,
            tc.tile_pool(name="t2xp", bufs=2) as t2xp,
            tc.tile_pool(name="t2p", bufs=2) as t2p,
            tc.tile_pool(name="t3p", bufs=2) as t3p,
            tc.tile_pool(name="top", bufs=2) as top,
            tc.tile_pool(name="gpp", bufs=3, space="PSUM") as gpp,
            tc.tile_pool(name="ttp", bufs=2, space="PSUM") as ttp,
        ):
            s2p = cst.tile([C, hp, WP], FP16, name="s2p")
            ident = cst.tile([128, 128], FP16, name="ident")
            make_identity(nc, ident)

            # Prefetch all stationary chunks on GpSimd (SWDGE) upfront.
            s1cs = []
            for ch in range(nch):
                s1c = s1p.tile(
                    [C, NBLK, NG, 128], FP16, name=f"s1c{ch}", tag=f"s1c{ch}"
                )
                nc.gpsimd.dma_start(
                    s1c[:, :, :, :], in1s[:, ch * NBLK : (ch + 1) * NBLK, :, :]
                )
                s1cs.append(s1c)

            # Load in2p in slices matching per-chunk consumption:
            # chunk ch needs padded rows [RCH*ch, RCH*ch + RCH + 8).
            row_hi = 0
            for ch in range(nch):
                need = min(RCH * (ch + 1) + 2 * PAD, hp) if ch < nch - 1 else hp
                if need > row_hi:
                    nc.gpsimd.dma_start(
                        s2p[:, row_hi:need, :], in2p[:, row_hi:need, :]
                    )
                    row_hi = need

            for ch in range(nch):
                y0 = ch * RCH
                s1c = s1cs[ch]

                # W[m | dy', v, blkg] fp16, blkg = 8*blk + g innermost
                wt = wp.tile([128, DYP, VP, NBG], FP16, name="wt", tag="wt")
                for blk in range(NBLK):
                    yb = y0 + BR * blk  # top row of block, in padded coords
                    for half in range(NG // 2):
                        gp = gpp.tile([128, 2, 512], FP32, name="gp", tag="gp")
                        for j in range(2):
                            g = 2 * half + j
                            nc.tensor.matmul(
                                gp[:, j, 0:NMM].rearrange(
                                    "p (dy v) -> p dy v", dy=DYP
                                ),
                                s1c[:, blk, g, :],
                                s2p[:, yb : yb + DYP, BU * g : BU * g + VP],
                                start=True,
                                stop=True,
                            )
                        # PSUM -> W (fp32 -> fp16); innermost j-pair contiguous
                        bg = NG * blk + 2 * half
                        dst = wt[:, :, :, bg : bg + 2]
                        src = gp[:, :, 0:NMM].rearrange(
                            "p j (dy v) -> p dy v j", dy=DYP
                        )
                        if (blk * 4 + half) % 16 < 9:
                            nc.scalar.activation(dst, src, COPY)
                        else:
                            nc.vector.tensor_copy(dst, src)

                # --- u-extract (x-shift): 16 DMAs on Sync, 576B runs ---
                # t2x[m | dy', dx, blkg] = wt[m | dy', u+dx, blkg], u = m%16
                t2x = t2xp.tile([128, DYP, D, NBG], FP16, name="t2x", tag="t2x")
                for u in range(BU):
                    nc.sync.dma_start(
                        t2x[u::BU, :, :, :],
                        wt[u::BU, :, u : u + D, :],
                    )

                # --- r-unshift (y-shift): 8 DMAs on GpSimd, 5.2KB runs ---
                # t2f[16r+u | dy, dx, blkg] = t2x[16r+u | r+dy, dx, blkg]
                t2f = t2p.tile([128, D, D, NBG], FP16, name="t2f", tag="t2f")
                for r in range(BR):
                    nc.gpsimd.dma_start(
                        t2f[BU * r : BU * r + BU, :, :, :],
                        t2x[BU * r : BU * r + BU, r : r + D, :, :],
                    )

                # --- reorder for PE transpose: t3[m | blkg, dy, dx] ---
                t3 = t3p.tile([128, NBG, D, D], FP16, name="t3", tag="t3")
                nc.vector.tensor_copy(
                    t3[:, :, :, :],
                    t2f[:, :, :, :].rearrange("p dy dx b -> p b dy dx"),
                )

                # --- PE transpose + drain (1/C scale, fp16) + store ---
                to = top.tile([K, RCH, W], FP16, name="to", tag="to")
                for blk in range(NBLK):
                    for hf in range(2):
                        tt = ttp.tile([K, 4, 128], FP16, name="tt", tag="tt")
                        for gi in range(4):
                            g = 4 * hf + gi
                            bg = NG * blk + g
                            nc.tensor.transpose(
                                tt[:, gi, :],
                                t3[:, bg, :, :].rearrange("p dy dx -> p (dy dx)"),
                                ident[:, :],
                            )
                        # to[k, 8blk+r, 16g+u] <- tt[k, gi, 16r+u]
                        dst = to[
                            :, BR * blk : BR * blk + BR, 64 * hf : 64 * hf + 64
                        ].rearrange("k r (g u) -> k r g u", g=4)
                        src = tt[:, :, :].rearrange("k g (r u) -> k r g u", r=BR)
                        if (blk * 2 + hf) % 2 == 0:
                            nc.scalar.activation(dst, src, COPY, scale=1.0 / C)
                        else:
                            nc.vector.tensor_scalar_mul(dst, src, 1.0 / C)

                # --- store: contiguous [81, RCH*W] fp16 block ---
                nc.scalar.dma_start(out[:, y0 : y0 + RCH, :], to[:, :, :])

    nc.compile()
    return nc


_cached = {}


def _get_nc(h: int):
    if h not in _cached:
        _cached[h] = build_bass(h)
    return _cached[h]


def _pad_in2(in2: np.ndarray) -> np.ndarray:
    # [C, h, W] fp16 -> [C, h+8, W+8] zero-padded, contiguous fp16
    return np.pad(
        in2.astype(np.float16), ((0, 0), (PAD, PAD), (PAD, PAD)), mode="constant"
    )


def _shuffle_in1(in1: np.ndarray) -> np.ndarray:
    # [C, h, W] -> [C, h//8, 8(g), 128(m)] with m = 16r+u,
    # y = 8*blk + r, x = 16*g + u.
    c, h, w = in1.shape
    a = in1.astype(np.float16).reshape(c, h // BR, BR, NG, BU)  # c,blk,r,g,u
    a = a.transpose(0, 1, 3, 2, 4)  # c, blk, g, r, u
    return np.ascontiguousarray(a.reshape(c, h // BR, NG, 128))


def kernel(**inputs: np.ndarray) -> np.ndarray:
    in1 = np.asarray(inputs["in1"], dtype=np.float32)
    in2 = np.asarray(inputs["in2"], dtype=np.float32)
    assert in1.shape == (B, C, H, W), in1.shape

    nc = _get_nc(H)
    in_maps = [
        {
            "in1s": _shuffle_in1(in1[b]),
            "in2p": np.ascontiguousarray(_pad_in2(in2[b])),
        }
        for b in range(B)
    ]
    res = run_bass_kernel_spmd(nc, in_maps, core_ids=list(range(N_CORES)))
    return np.stack([r["out"] for r in res.results], axis=0).astype(np.float32)


# revision 8
# speedup vs baseline: 4.8505x; 1.0500x over previous
"""Correlation cost-volume kernel for Trainium2 (Bass/Tile), v3.

Problem: in1, in2: [B=8, C=128, H=96, W=128] fp32.
Output: [B, 81, H, W] where out[b, dy*9+dx, y, x] =
    mean_c( in1[b,c,y,x] * in2_pad[b,c,y+dy,x+dx] ),
with in2 zero-padded by 4 in both spatial dims (max_displacement=4).

Data-parallel over batch (one sample per NeuronCore), fp16 compute.

Pipeline per 32-row chunk (4 row-blocks x 8 col-groups of 8x16 pixels,
m = 16r+u):
  1. matmul per block: stationary in1[:, 8rows x 16px] (M=128), moving
     in2p[:, 16 dy', 24 v] (N=384) -> psum[m, dy', v].
  2. ACT/DVE drain psum -> W[m | dy'16, v24, blkg32] fp16 (blkg = 8blk+g
     innermost so the extraction DMAs below get 576B contiguous runs).
  3. u-extract (x-shift): 16 DMAs on Sync (one per u, partitions u::16):
     t2x[m | dy', dx, blkg] = W[m | dy', u+dx, blkg]; src/dst runs 576B+.
  4. r-unshift (y-shift): 8 DMAs on GpSimd (one per r, partitions
     16r..16r+16): t2f[m | dy, dx, blkg] = t2x[m | r+dy, dx, blkg]; one
     contiguous 5.2KB run per partition.
  5. DVE reorder t2f -> t3[m | blkg, 81] so the PE transpose stationary
     has a single contiguous free dim.
  6. PE transpose per block-group: tt[k=81, m] <- t3[:, bg, :].
  7. ACT/DVE drain with 1/C scale to fp16 staging, one contiguous store
     per chunk; host upcasts to fp32.

v3 vs v2: u-first extraction (1.18+0.66 MB/chunk vs 1.77+0.66), Scalar
does no DMAs (was starving the psum drains), all stationary loads are
prefetched upfront, in2p load slices match per-chunk consumption so
chunk-0 matmuls start ~6us in, PSUM pools 3x2-bank mm tiles + 2x1-bank
transpose tiles.
"""

import numpy as np

import concourse.bass as bass
import concourse.mybir as mybir
from concourse import bacc
from concourse.bass_utils import run_bass_kernel_spmd
from concourse.masks import make_identity
from concourse.tile import TileContext

B = 8
C = 128
H = 96
W = 128
D = 9  # 2*max_disp + 1
K = D * D  # 81 output channels
PAD = 4
WP = W + 2 * PAD  # 136
FP32 = mybir.dt.float32
FP16 = mybir.dt.float16

N_CORES = 8
RCH = 32  # rows per chunk
BR = 8  # block rows
BU = 16  # block cols
NBLK = RCH // BR  # 4 row-blocks per chunk
NG = W // BU  # 8 col-groups
NBG = NBLK * NG  # 32 pixel-blocks per chunk
DYP = BR + 2 * PAD  # 16 dy' values per block
VP = BU + 2 * PAD  # 24 v values per group
NMM = DYP * VP  # 384 matmul free size
COPY = mybir.ActivationFunctionType.Copy


def build_bass(h: int = H):
    """Build the per-core Bass program for a [C, h, W] sample."""
    hp = h + 2 * PAD
    nch = h // RCH
    assert h % RCH == 0
    nc = bacc.Bacc(None, target_bir_lowering=False)
    # in1s is host-shuffled to [C, blk_total, g, m] with m = 16r+u,
    # y = 8*blk + r, x = 16*g + u (stationary needs one free dim).
    in1s = nc.dram_tensor("in1s", [C, h // BR, NG, 128], FP16, kind="ExternalInput")
    # in2p is host-padded: [C, h+8, W+8] with zeros in the 4-wide borders.
    in2p = nc.dram_tensor("in2p", [C, hp, WP], FP16, kind="ExternalInput")
    out = nc.dram_tensor("out", [K, h, W], FP16, kind="ExternalOutput")

    with TileContext(nc) as tc:
        with (
            tc.tile_pool(name="cst", bufs=1) as cst,
            tc.tile_pool(name="s1p", bufs=3) as s1p,
            tc.tile_pool(name="wp", bufs=2) as wp,
            tc.tile_pool(name="t2xp", bufs=2) as t2xp,
            tc.tile_pool(name="t2p", bufs=2) as t2p,
            tc.tile_pool(name="t3p", bufs=2) as t3p,
            tc.tile_pool(name="top", bufs=2) as top,
            tc.tile_pool(name="gpp", bufs=2, space="PSUM") as gpp,
            tc.tile_pool(name="ttp", bufs=4, space="PSUM") as ttp,
        ):
            s2p = cst.tile([C, hp, WP], FP16, name="s2p")
            ident = cst.tile([128, 128], FP16, name="ident")

            # Interleave input loads so chunk-0 compute starts earliest:
            # s1c0, in2p rows for blocks 0-1, rows for blocks 2-3, s1c1, ...
            s1cs = []
            row_hi = 0

            def _load_s1c(ch):
                s1c = s1p.tile(
                    [C, NBLK, NG, 128], FP16, name=f"s1c{ch}", tag=f"s1c{ch}"
                )
                nc.gpsimd.dma_start(
                    s1c[:, :, :, :], in1s[:, ch * NBLK : (ch + 1) * NBLK, :, :]
                )
                s1cs.append(s1c)

            def _load_s2p(need):
                nonlocal row_hi
                need = min(need, hp)
                if need > row_hi:
                    nc.gpsimd.dma_start(
                        s2p[:, row_hi:need, :], in2p[:, row_hi:need, :]
                    )
                    row_hi = need

            _load_s1c(0)
            _load_s2p(2 * BR + 2 * PAD)  # rows for chunk-0 blocks 0-1
            _load_s2p(RCH + 2 * PAD)  # rest of chunk 0
            for ch in range(1, nch):
                _load_s1c(ch)
                _load_s2p(RCH * (ch + 1) + 2 * PAD if ch < nch - 1 else hp)

            make_identity(nc, ident)

            for ch in range(nch):
                y0 = ch * RCH
                s1c = s1cs[ch]

                # W[m | dy', v, blkg] fp16, blkg = 8*blk + g innermost
                wt = wp.tile([128, DYP, VP, NBG], FP16, name="wt", tag="wt")
                for blk in range(NBLK):
                    yb = y0 + BR * blk  # top row of block, in padded coords
                    for half in range(NG // 2):
                        gp = gpp.tile([128, 2, 512], FP32, name="gp", tag="gp")
                        for j in range(2):
                            g = 2 * half + j
                            nc.tensor.matmul(
                                gp[:, j, 0:NMM].rearrange(
                                    "p (dy v) -> p dy v", dy=DYP
                                ),
                                s1c[:, blk, g, :],
                                s2p[:, yb : yb + DYP, BU * g : BU * g + VP],
                                start=True,
                                stop=True,
                            )
                        # PSUM -> W (fp32 -> fp16); innermost j-pair contiguous
                        bg = NG * blk + 2 * half
                        dst = wt[:, :, :, bg : bg + 2]
                        src = gp[:, :, 0:NMM].rearrange(
                            "p j (dy v) -> p dy v j", dy=DYP
                        )
                        if (blk * 4 + half) % 16 < 9:
                            nc.scalar.activation(dst, src, COPY)
                        else:
                            nc.vector.tensor_copy(dst, src)

                # --- u-extract (x-shift): 16 DMAs on Sync, 576B runs ---
                # t2x[m | dy', dx, blkg] = wt[m | dy', u+dx, blkg], u = m%16
                t2x = t2xp.tile([128, DYP, D, NBG], FP16, name="t2x", tag="t2x")
                for u in range(BU):
                    nc.sync.dma_start(
                        t2x[u::BU, :, :, :],
                        wt[u::BU, :, u : u + D, :],
                    )

                # --- r-unshift (y-shift): 8 DMAs on GpSimd, 5.2KB runs ---
                # t2f[16r+u | dy, dx, blkg] = t2x[16r+u | r+dy, dx, blkg]
                t2f = t2p.tile([128, D, D, NBG], FP16, name="t2f", tag="t2f")
                for r in range(BR):
                    nc.gpsimd.dma_start(
                        t2f[BU * r : BU * r + BU, :, :, :],
                        t2x[BU * r : BU * r + BU, r : r + D, :, :],
                    )

                # --- reorder for PE transpose: t3[m | blkg, dy, dx] ---
                t3 = t3p.tile([128, NBG, D, D], FP16, name="t3", tag="t3")
                nc.vector.tensor_copy(
                    t3[:, :, :, :],
                    t2f[:, :, :, :].rearrange("p dy dx b -> p b dy dx"),
                )

                # --- PE transpose + drain (1/C scale, fp16) + store ---
                to = top.tile([K, RCH, W], FP16, name="to", tag="to")
                for blk in range(NBLK):
                    for hf in range(2):
                        tt = ttp.tile([K, 4, 128], FP16, name="tt", tag="tt")
                        for gi in range(4):
                            g = 4 * hf + gi
                            bg = NG * blk + g
                            nc.tensor.transpose(
                                tt[:, gi, :],
                                t3[:, bg, :, :].rearrange("p dy dx -> p (dy dx)"),
                                ident[:, :],
                            )
                        # to[k, 8blk+r, 16g+u] <- tt[k, gi, 16r+u]
                        dst = to[
                            :, BR * blk : BR * blk + BR, 64 * hf : 64 * hf + 64
                        ].rearrange("k r (g u) -> k r g u", g=4)
                        src = tt[:, :, :].rearrange("k g (r u) -> k r g u", r=BR)
                        if (blk * 2 + hf) % 2 == 0:
                            nc.scalar.activation(dst, src, COPY, scale=1.0 / C)
                        else:
                            nc.vector.tensor_scalar_mul(dst, src, 1.0 / C)

                # --- store: contiguous [81, RCH*W] fp16 block ---
                nc.scalar.dma_start(out[:, y0 : y0 + RCH, :], to[:, :, :])

    nc.compile()
    return nc


_cached = {}


def _get_nc(h: int):
    if h not in _cached:
        _cached[h] = build_bass(h)
    return _cached[h]


def _pad_in2(in2: np.ndarray) -> np.ndarray:
    # [C, h, W] fp16 -> [C, h+8, W+8] zero-padded, contiguous fp16
    return np.pad(
        in2.astype(np.float16), ((0, 0), (PAD, PAD), (PAD, PAD)), mode="constant"
    )


def _shuffle_in1(in1: np.ndarray) -> np.ndarray:
    # [C, h, W] -> [C, h//8, 8(g), 128(m)] with m = 16r+u,
    # y = 8*blk + r, x = 16*g + u.
    c, h, w = in1.shape
    a = in1.astype(np.float16).reshape(c, h // BR, BR, NG, BU)  # c,blk,r,g,u
    a = a.transpose(0, 1, 3, 2, 4)  # c, blk, g, r, u
    return np.ascontiguousarray(a.reshape(c, h // BR, NG, 128))


def kernel(**inputs: np.ndarray) -> np.ndarray:
    in1 = np.asarray(inputs["in1"], dtype=np.float32)
    in2 = np.asarray(inputs["in2"], dtype=np.float32)
    assert in1.shape == (B, C, H, W), in1.shape

    nc = _get_nc(H)
    in_maps = [
        {
            "in1s": _shuffle_in1(in1[b]),
            "in2p": np.ascontiguousarray(_pad_in2(in2[b])),
        }
        for b in range(B)
    ]
    res = run_bass_kernel_spmd(nc, in_maps, core_ids=list(range(N_CORES)))
    return np.stack([r["out"] for r in res.results], axis=0).astype(np.float32)


# revision 9
# speedup vs baseline: 5.5377x; 1.1417x over previous
"""Correlation cost-volume kernel for Trainium2 (Bass/Tile), v3.

Problem: in1, in2: [B=8, C=128, H=96, W=128] fp32.
Output: [B, 81, H, W] where out[b, dy*9+dx, y, x] =
    mean_c( in1[b,c,y,x] * in2_pad[b,c,y+dy,x+dx] ),
with in2 zero-padded by 4 in both spatial dims (max_displacement=4).

Data-parallel over batch (one sample per NeuronCore), fp16 compute.

Pipeline per 32-row chunk (4 row-blocks x 8 col-groups of 8x16 pixels,
m = 16r+u):
  1. matmul per block: stationary in1[:, 8rows x 16px] (M=128), moving
     in2p[:, 16 dy', 24 v] (N=384) -> psum[m, dy', v].
  2. ACT/DVE drain psum -> W[m | dy'16, v24, blkg32] fp16 (blkg = 8blk+g
     innermost so the extraction DMAs below get 576B contiguous runs).
  3. u-extract (x-shift): 16 DMAs on Sync (one per u, partitions u::16):
     t2x[m | dy', dx, blkg] = W[m | dy', u+dx, blkg]; src/dst runs 576B+.
  4. r-unshift (y-shift): 8 DMAs on GpSimd (one per r, partitions
     16r..16r+16): t2f[m | dy, dx, blkg] = t2x[m | r+dy, dx, blkg]; one
     contiguous 5.2KB run per partition.
  5. DVE reorder t2f -> t3[m | blkg, 81] so the PE transpose stationary
     has a single contiguous free dim.
  6. PE transpose per block-group: tt[k=81, m] <- t3[:, bg, :].
  7. ACT/DVE drain with 1/C scale to fp16 staging, one contiguous store
     per chunk; host upcasts to fp32.

v3 vs v2: u-first extraction (1.18+0.66 MB/chunk vs 1.77+0.66), Scalar
does no DMAs (was starving the psum drains), all stationary loads are
prefetched upfront, in2p load slices match per-chunk consumption so
chunk-0 matmuls start ~6us in, PSUM pools 3x2-bank mm tiles + 2x1-bank
transpose tiles.
"""

import numpy as np

import concourse.bass as bass
import concourse.mybir as mybir
from concourse import bacc
from concourse.bass_utils import run_bass_kernel_spmd
from concourse.masks import make_identity
from concourse.tile import TileContext

B = 8
C = 128
H = 96
W = 128
D = 9  # 2*max_disp + 1
K = D * D  # 81 output channels
PAD = 4
WP = W + 2 * PAD  # 136
FP32 = mybir.dt.float32
FP16 = mybir.dt.float16

N_CORES = 8
RCH = 32  # rows per chunk
BR = 8  # block rows
BU = 16  # block cols
NBLK = RCH // BR  # 4 row-blocks per chunk
NG = W // BU  # 8 col-groups
NBG = NBLK * NG  # 32 pixel-blocks per chunk
DYP = BR + 2 * PAD  # 16 dy' values per block
VP = BU + 2 * PAD  # 24 v values per group
NMM = DYP * VP  # 384 matmul free size
COPY = mybir.ActivationFunctionType.Copy


def build_bass(h: int = H):
    """Build the per-core Bass program for a [C, h, W] sample."""
    hp = h + 2 * PAD
    nch = h // RCH
    assert h % RCH == 0
    nc = bacc.Bacc(None, target_bir_lowering=False)
    # in1s is host-shuffled to [C, blk_total, g, m] with m = 16r+u,
    # y = 8*blk + r, x = 16*g + u (stationary needs one free dim).
    in1s = nc.dram_tensor("in1s", [C, h // BR, NG, 128], FP16, kind="ExternalInput")
    # in2p is host-padded: [C, h+8, W+8] with zeros in the 4-wide borders.
    in2p = nc.dram_tensor("in2p", [C, hp, WP], FP16, kind="ExternalInput")
    out = nc.dram_tensor("out", [K, h, W], FP16, kind="ExternalOutput")

    with TileContext(nc) as tc:
        with (
            tc.tile_pool(name="cst", bufs=1) as cst,
            tc.tile_pool(name="s1p", bufs=3) as s1p,
            tc.tile_pool(name="wp", bufs=2) as wp,
            tc.tile_pool(name="t2xp", bufs=2) as t2xp,
            tc.tile_pool(name="t2p", bufs=2) as t2p,
            tc.tile_pool(name="t3p", bufs=2) as t3p,
            tc.tile_pool(name="top", bufs=2) as top,
            tc.tile_pool(name="gpp", bufs=2, space="PSUM") as gpp,
            tc.tile_pool(name="ttp", bufs=4, space="PSUM") as ttp,
        ):
            s2p = cst.tile([C, hp, WP], FP16, name="s2p")
            ident = cst.tile([128, 128], FP16, name="ident")

            # Interleave input loads so chunk-0 compute starts earliest:
            # s1c0, in2p rows for blocks 0-1, rows for blocks 2-3, s1c1, ...
            s1cs = []
            row_hi = 0

            def _load_s1c(ch):
                s1c = s1p.tile(
                    [C, NBLK, NG, 128], FP16, name=f"s1c{ch}", tag=f"s1c{ch}"
                )
                nc.gpsimd.dma_start(
                    s1c[:, :, :, :], in1s[:, ch * NBLK : (ch + 1) * NBLK, :, :]
                )
                s1cs.append(s1c)

            def _load_s2p(need):
                nonlocal row_hi
                need = min(need, hp)
                if need > row_hi:
                    nc.gpsimd.dma_start(
                        s2p[:, row_hi:need, :], in2p[:, row_hi:need, :]
                    )
                    row_hi = need

            _load_s1c(0)
            _load_s2p(2 * BR + 2 * PAD)  # rows for chunk-0 blocks 0-1
            _load_s2p(RCH + 2 * PAD)  # rest of chunk 0
            for ch in range(1, nch):
                _load_s1c(ch)
                _load_s2p(RCH * (ch + 1) + 2 * PAD if ch < nch - 1 else hp)

            make_identity(nc, ident)

            def head(ch):
                """MMs + psum drains + extraction DMAs for chunk ch."""
                y0 = ch * RCH
                s1c = s1cs[ch]

                # W[m | dy', v, blkg] fp16, blkg = 8*blk + g innermost
                wt = wp.tile([128, DYP, VP, NBG], FP16, name="wt", tag="wt")
                for blk in range(NBLK):
                    yb = y0 + BR * blk  # top row of block, in padded coords
                    for half in range(NG // 2):
                        gp = gpp.tile([128, 2, 512], FP32, name="gp", tag="gp")
                        for j in range(2):
                            g = 2 * half + j
                            nc.tensor.matmul(
                                gp[:, j, 0:NMM].rearrange(
                                    "p (dy v) -> p dy v", dy=DYP
                                ),
                                s1c[:, blk, g, :],
                                s2p[:, yb : yb + DYP, BU * g : BU * g + VP],
                                start=True,
                                stop=True,
                            )
                        # PSUM -> W (fp32 -> fp16); innermost j-pair contiguous
                        bg = NG * blk + 2 * half
                        dst = wt[:, :, :, bg : bg + 2]
                        src = gp[:, :, 0:NMM].rearrange(
                            "p j (dy v) -> p dy v j", dy=DYP
                        )
                        i = blk * 4 + half
                        if i % 2 == 1 and i < 14:
                            nc.vector.tensor_copy(dst, src)
                        else:
                            nc.scalar.activation(dst, src, COPY)

                # --- u-extract (x-shift): 16 DMAs on Sync, 576B runs ---
                # t2x[m | dy', dx, blkg] = wt[m | dy', u+dx, blkg], u = m%16
                t2x = t2xp.tile([128, DYP, D, NBG], FP16, name="t2x", tag="t2x")
                for u in range(BU):
                    nc.sync.dma_start(
                        t2x[u::BU, :, :, :],
                        wt[u::BU, :, u : u + D, :],
                    )

                # --- r-unshift (y-shift): 8 DMAs on GpSimd, 5.2KB runs ---
                # t2f[16r+u | dy, dx, blkg] = t2x[16r+u | r+dy, dx, blkg]
                t2f = t2p.tile([128, D, D, NBG], FP16, name="t2f", tag="t2f")
                for r in range(BR):
                    nc.gpsimd.dma_start(
                        t2f[BU * r : BU * r + BU, :, :, :],
                        t2x[BU * r : BU * r + BU, r : r + D, :, :],
                    )
                return t2f

            def tail(ch, t2f):
                """Reorder + PE transpose + scaled drain + store for chunk ch."""
                y0 = ch * RCH
                t3 = t3p.tile([128, NBG, D, D], FP16, name="t3", tag="t3")
                nc.vector.tensor_copy(
                    t3[:, :, :, :],
                    t2f[:, :, :, :].rearrange("p dy dx b -> p b dy dx"),
                )

                to = top.tile([K, RCH, W], FP16, name="to", tag="to")
                for blk in range(NBLK):
                    for hf in range(2):
                        tt = ttp.tile([K, 4, 128], FP16, name="tt", tag="tt")
                        for gi in range(4):
                            g = 4 * hf + gi
                            bg = NG * blk + g
                            nc.tensor.transpose(
                                tt[:, gi, :],
                                t3[:, bg, :, :].rearrange("p dy dx -> p (dy dx)"),
                                ident[:, :],
                            )
                        # to[k, 8blk+r, 16g+u] <- tt[k, gi, 16r+u]
                        dst = to[
                            :, BR * blk : BR * blk + BR, 64 * hf : 64 * hf + 64
                        ].rearrange("k r (g u) -> k r g u", g=4)
                        src = tt[:, :, :].rearrange("k g (r u) -> k r g u", r=BR)
                        if (blk * 2 + hf) % 2 == 0:
                            nc.scalar.activation(dst, src, COPY, scale=1.0 / C)
                        else:
                            nc.vector.tensor_scalar_mul(dst, src, 1.0 / C)

                # --- store: contiguous [81, RCH*W] fp16 block ---
                nc.sync.dma_start(out[:, y0 : y0 + RCH, :], to[:, :, :])

            # Software-pipelined emission: chunk ch's tail is emitted after
            # chunk ch+1's head so each engine's static schedule interleaves
            # the extraction chain of one chunk with the compute of the next.
            prev = None
            for ch in range(nch):
                t2f = head(ch)
                if prev is not None:
                    tail(ch - 1, prev)
                prev = t2f
            tail(nch - 1, prev)

    nc.compile()
    return nc


_cached = {}


def _get_nc(h: int):
    if h not in _cached:
        _cached[h] = build_bass(h)
    return _cached[h]


def _pad_in2(in2: np.ndarray) -> np.ndarray:
    # [C, h, W] fp16 -> [C, h+8, W+8] zero-padded, contiguous fp16
    return np.pad(
        in2.astype(np.float16), ((0, 0), (PAD, PAD), (PAD, PAD)), mode="constant"
    )


def _shuffle_in1(in1: np.ndarray) -> np.ndarray:
    # [C, h, W] -> [C, h//8, 8(g), 128(m)] with m = 16r+u,
    # y = 8*blk + r, x = 16*g + u.
    c, h, w = in1.shape
    a = in1.astype(np.float16).reshape(c, h // BR, BR, NG, BU)  # c,blk,r,g,u
    a = a.transpose(0, 1, 3, 2, 4)  # c, blk, g, r, u
    return np.ascontiguousarray(a.reshape(c, h // BR, NG, 128))


def kernel(**inputs: np.ndarray) -> np.ndarray:
    in1 = np.asarray(inputs["in1"], dtype=np.float32)
    in2 = np.asarray(inputs["in2"], dtype=np.float32)
    assert in1.shape == (B, C, H, W), in1.shape

    nc = _get_nc(H)
    in_maps = [
        {
            "in1s": _shuffle_in1(in1[b]),
            "in2p": np.ascontiguousarray(_pad_in2(in2[b])),
        }
        for b in range(B)
    ]
    res = run_bass_kernel_spmd(nc, in_maps, core_ids=list(range(N_CORES)))
    return np.stack([r["out"] for r in res.results], axis=0).astype(np.float32)


# revision 13
# speedup vs baseline: 5.7945x; 1.0464x over previous
"""Correlation cost-volume kernel for Trainium2 (Bass/Tile), v3.

Problem: in1, in2: [B=8, C=128, H=96, W=128] fp32.
Output: [B, 81, H, W] where out[b, dy*9+dx, y, x] =
    mean_c( in1[b,c,y,x] * in2_pad[b,c,y+dy,x+dx] ),
with in2 zero-padded by 4 in both spatial dims (max_displacement=4).

Data-parallel over batch (one sample per NeuronCore), fp16 compute.

Pipeline per 32-row chunk (4 row-blocks x 8 col-groups of 8x16 pixels,
m = 16r+u):
  1. matmul per block: stationary in1[:, 8rows x 16px] (M=128), moving
     in2p[:, 16 dy', 24 v] (N=384) -> psum[m, dy', v].
  2. ACT/DVE drain psum -> W[m | dy'16, v24, blkg32] fp16 (blkg = 8blk+g
     innermost so the extraction DMAs below get 576B contiguous runs).
  3. u-extract (x-shift): 16 DMAs on Sync (one per u, partitions u::16):
     t2x[m | dy', dx, blkg] = W[m | dy', u+dx, blkg]; src/dst runs 576B+.
  4. r-unshift (y-shift): 8 DMAs on GpSimd (one per r, partitions
     16r..16r+16): t2f[m | dy, dx, blkg] = t2x[m | r+dy, dx, blkg]; one
     contiguous 5.2KB run per partition.
  5. DVE reorder t2f -> t3[m | blkg, 81] so the PE transpose stationary
     has a single contiguous free dim.
  6. PE transpose per block-group: tt[k=81, m] <- t3[:, bg, :].
  7. ACT/DVE drain with 1/C scale to fp16 staging, one contiguous store
     per chunk; host upcasts to fp32.

v3 vs v2: u-first extraction (1.18+0.66 MB/chunk vs 1.77+0.66), Scalar
does no DMAs (was starving the psum drains), all stationary loads are
prefetched upfront, in2p load slices match per-chunk consumption so
chunk-0 matmuls start ~6us in, PSUM pools 3x2-bank mm tiles + 2x1-bank
transpose tiles.
"""

import numpy as np

import concourse.bass as bass
import concourse.mybir as mybir
from concourse import bacc
from concourse.bass_utils import run_bass_kernel_spmd
from concourse.masks import make_identity
from concourse.tile import TileContext

B = 8
C = 128
H = 96
W = 128
D = 9  # 2*max_disp + 1
K = D * D  # 81 output channels
PAD = 4
WP = W + 2 * PAD  # 136
FP32 = mybir.dt.float32
FP16 = mybir.dt.float16

N_CORES = 8
RCH = 32  # rows per chunk
BR = 8  # block rows
BU = 16  # block cols
NBLK = RCH // BR  # 4 row-blocks per chunk
NG = W // BU  # 8 col-groups
NBG = NBLK * NG  # 32 pixel-blocks per chunk
DYP = BR + 2 * PAD  # 16 dy' values per block
VP = BU + 2 * PAD  # 24 v values per group
NMM = DYP * VP  # 384 matmul free size
COPY = mybir.ActivationFunctionType.Copy


def build_bass(h: int = H):
    """Build the per-core Bass program for a [C, h, W] sample."""
    hp = h + 2 * PAD
    nch = h // RCH
    assert h % RCH == 0
    nc = bacc.Bacc(None, target_bir_lowering=False)
    # in1s is host-shuffled to [C, blk_total, g, m] with m = 16r+u,
    # y = 8*blk + r, x = 16*g + u (stationary needs one free dim).
    in1s = nc.dram_tensor("in1s", [C, h // BR, NG, 128], FP16, kind="ExternalInput")
    # in2p is host-padded: [C, h+8, W+8] with zeros in the 4-wide borders.
    in2p = nc.dram_tensor("in2p", [C, hp, WP], FP16, kind="ExternalInput")
    out = nc.dram_tensor("out", [K, h, W], FP16, kind="ExternalOutput")

    with TileContext(nc) as tc:
        with (
            tc.tile_pool(name="cst", bufs=1) as cst,
            tc.tile_pool(name="s1p", bufs=3) as s1p,
            tc.tile_pool(name="wp", bufs=2) as wp,
            tc.tile_pool(name="t2xp", bufs=2) as t2xp,
            tc.tile_pool(name="t2p", bufs=2) as t2p,
            tc.tile_pool(name="t3p", bufs=2) as t3p,
            tc.tile_pool(name="top", bufs=2) as top,
            tc.tile_pool(name="gpp", bufs=2, space="PSUM") as gpp,
            tc.tile_pool(name="ttp", bufs=4, space="PSUM") as ttp,
        ):
            s2p = cst.tile([C, hp, WP], FP16, name="s2p")
            ident = cst.tile([128, 128], FP16, name="ident")

            # Interleave input loads so chunk-0 compute starts earliest:
            # s1c0, in2p rows for blocks 0-1, rows for blocks 2-3, s1c1, ...
            s1cs = []
            row_hi = 0

            def _load_s1c(ch):
                s1c = s1p.tile(
                    [C, NBLK, NG, 128], FP16, name=f"s1c{ch}", tag=f"s1c{ch}"
                )
                # chunk 0 on Sync so its transfer overlaps the first in2p
                # slice on the GpSimd ring
                eng = nc.sync if ch == 0 else nc.gpsimd
                eng.dma_start(
                    s1c[:, :, :, :], in1s[:, ch * NBLK : (ch + 1) * NBLK, :, :]
                )
                s1cs.append(s1c)

            def _load_s2p(need):
                nonlocal row_hi
                need = min(need, hp)
                if need > row_hi:
                    nc.gpsimd.dma_start(
                        s2p[:, row_hi:need, :], in2p[:, row_hi:need, :]
                    )
                    row_hi = need

            _load_s1c(0)
            _load_s2p(2 * BR + 2 * PAD)  # rows for chunk-0 blocks 0-1
            _load_s2p(RCH + 2 * PAD)  # rest of chunk 0
            for ch in range(1, nch):
                _load_s1c(ch)
                _load_s2p(RCH * (ch + 1) + 2 * PAD if ch < nch - 1 else hp)

            make_identity(nc, ident)

            def head(ch):
                """MMs + psum drains + extraction DMAs for chunk ch."""
                y0 = ch * RCH
                s1c = s1cs[ch]

                # W[m | dy', v, blkg] fp16, blkg = 8*blk + g innermost
                wt = wp.tile([128, DYP, VP, NBG], FP16, name="wt", tag="wt")
                for blk in range(NBLK):
                    yb = y0 + BR * blk  # top row of block, in padded coords
                    for half in range(NG // 2):
                        gp = gpp.tile([128, 2, 512], FP32, name="gp", tag="gp")
                        for j in range(2):
                            g = 2 * half + j
                            nc.tensor.matmul(
                                gp[:, j, 0:NMM].rearrange(
                                    "p (dy v) -> p dy v", dy=DYP
                                ),
                                s1c[:, blk, g, :],
                                s2p[:, yb : yb + DYP, BU * g : BU * g + VP],
                                start=True,
                                stop=True,
                            )
                        # PSUM -> W (fp32 -> fp16); innermost j-pair contiguous
                        bg = NG * blk + 2 * half
                        dst = wt[:, :, :, bg : bg + 2]
                        src = gp[:, :, 0:NMM].rearrange(
                            "p j (dy v) -> p dy v j", dy=DYP
                        )
                        i = blk * 4 + half
                        if i % 2 == 1 and i < 14:
                            nc.vector.tensor_copy(dst, src)
                        else:
                            nc.scalar.activation(dst, src, COPY)

                # --- u-extract (x-shift): 16 DMAs on Sync, 576B runs ---
                # t2x[m | dy', dx, blkg] = wt[m | dy', u+dx, blkg], u = m%16
                t2x = t2xp.tile([128, DYP, D, NBG], FP16, name="t2x", tag="t2x")
                for u in range(BU):
                    nc.sync.dma_start(
                        t2x[u::BU, :, :, :],
                        wt[u::BU, :, u : u + D, :],
                    )

                # --- r-unshift (y-shift): 8 DMAs, 5.2KB runs ---
                # t2f[16r+u | dy, dx, blkg] = t2x[16r+u | r+dy, dx, blkg]
                # Last chunk: Sync (it is idle by then and skips the SWDGE
                # queue-drain stall on the exposed final tail).
                t2f = t2p.tile([128, D, D, NBG], FP16, name="t2f", tag="t2f")
                eng2 = nc.sync if ch == nch - 1 else nc.gpsimd
                for r in range(BR):
                    eng2.dma_start(
                        t2f[BU * r : BU * r + BU, :, :, :],
                        t2x[BU * r : BU * r + BU, r : r + D, :, :],
                    )
                return t2f

            def tail(ch, t2f):
                """Reorder + PE transpose + scaled drain + store for chunk ch."""
                y0 = ch * RCH
                last = ch == nch - 1
                t3 = t3p.tile([128, NBG, D, D], FP16, name="t3", tag="t3")
                if last:
                    # split across both engines: latency is exposed here
                    nc.vector.tensor_copy(
                        t3[:, 0 : NBG // 2, :, :],
                        t2f[:, :, :, 0 : NBG // 2].rearrange(
                            "p dy dx b -> p b dy dx"
                        ),
                    )
                    nc.scalar.activation(
                        t3[:, NBG // 2 :, :, :],
                        t2f[:, :, :, NBG // 2 :].rearrange(
                            "p dy dx b -> p b dy dx"
                        ),
                        COPY,
                    )
                else:
                    nc.vector.tensor_copy(
                        t3[:, :, :, :],
                        t2f[:, :, :, :].rearrange("p dy dx b -> p b dy dx"),
                    )

                to = top.tile([K, RCH, W], FP16, name="to", tag="to")
                for blk in range(NBLK):
                    for hf in range(2):
                        tt = ttp.tile([K, 4, 128], FP16, name="tt", tag="tt")
                        for gi in range(4):
                            g = 4 * hf + gi
                            bg = NG * blk + g
                            nc.tensor.transpose(
                                tt[:, gi, :],
                                t3[:, bg, :, :].rearrange("p dy dx -> p (dy dx)"),
                                ident[:, :],
                            )
                        # to[k, 8blk+r, 16g+u] <- tt[k, gi, 16r+u]
                        dst = to[
                            :, BR * blk : BR * blk + BR, 64 * hf : 64 * hf + 64
                        ].rearrange("k r (g u) -> k r g u", g=4)
                        src = tt[:, :, :].rearrange("k g (r u) -> k r g u", r=BR)
                        if (blk * 2 + hf) % 2 == 0:
                            nc.scalar.activation(dst, src, COPY, scale=1.0 / C)
                        else:
                            nc.vector.tensor_scalar_mul(dst, src, 1.0 / C)
                    if last and blk == NBLK // 2 - 1:
                        # pipeline the exposed final store with the drains
                        nc.sync.dma_start(
                            out[:, y0 : y0 + RCH // 2, :],
                            to[:, 0 : RCH // 2, :],
                        )

                # --- store: contiguous fp16 block ---
                if last:
                    nc.sync.dma_start(
                        out[:, y0 + RCH // 2 : y0 + RCH, :],
                        to[:, RCH // 2 :, :],
                    )
                else:
                    nc.sync.dma_start(out[:, y0 : y0 + RCH, :], to[:, :, :])

            # Software-pipelined emission: chunk ch's tail is emitted after
            # chunk ch+1's head so each engine's static schedule interleaves
            # the extraction chain of one chunk with the compute of the next.
            prev = None
            for ch in range(nch):
                t2f = head(ch)
                if prev is not None:
                    tail(ch - 1, prev)
                prev = t2f
            tail(nch - 1, prev)

    nc.compile()
    return nc


_cached = {}


def _get_nc(h: int):
    if h not in _cached:
        _cached[h] = build_bass(h)
    return _cached[h]


def _pad_in2(in2: np.ndarray) -> np.ndarray:
    # [C, h, W] fp16 -> [C, h+8, W+8] zero-padded, contiguous fp16
    return np.pad(
        in2.astype(np.float16), ((0, 0), (PAD, PAD), (PAD, PAD)), mode="constant"
    )


def _shuffle_in1(in1: np.ndarray) -> np.ndarray:
    # [C, h, W] -> [C, h//8, 8(g), 128(m)] with m = 16r+u,
    # y = 8*blk + r, x = 16*g + u.
    c, h, w = in1.shape
    a = in1.astype(np.float16).reshape(c, h // BR, BR, NG, BU)  # c,blk,r,g,u
    a = a.transpose(0, 1, 3, 2, 4)  # c, blk, g, r, u
    return np.ascontiguousarray(a.reshape(c, h // BR, NG, 128))


def kernel(**inputs: np.ndarray) -> np.ndarray:
    in1 = np.asarray(inputs["in1"], dtype=np.float32)
    in2 = np.asarray(inputs["in2"], dtype=np.float32)
    assert in1.shape == (B, C, H, W), in1.shape

    nc = _get_nc(H)
    in_maps = [
        {
            "in1s": _shuffle_in1(in1[b]),
            "in2p": np.ascontiguousarray(_pad_in2(in2[b])),
        }
        for b in range(B)
    ]
    res = run_bass_kernel_spmd(nc, in_maps, core_ids=list(range(N_CORES)))
    return np.stack([r["out"] for r in res.results], axis=0).astype(np.float32)


# revision 17
# speedup vs baseline: 6.0701x; 1.0476x over previous
"""Correlation cost-volume kernel for Trainium2 (Bass/Tile), v3.

Problem: in1, in2: [B=8, C=128, H=96, W=128] fp32.
Output: [B, 81, H, W] where out[b, dy*9+dx, y, x] =
    mean_c( in1[b,c,y,x] * in2_pad[b,c,y+dy,x+dx] ),
with in2 zero-padded by 4 in both spatial dims (max_displacement=4).

Data-parallel over batch (one sample per NeuronCore), fp16 compute.

Pipeline per 32-row chunk (4 row-blocks x 8 col-groups of 8x16 pixels,
m = 16r+u):
  1. matmul per block: stationary in1[:, 8rows x 16px] (M=128), moving
     in2p[:, 16 dy', 24 v] (N=384) -> psum[m, dy', v].
  2. ACT/DVE drain psum -> W[m | dy'16, v24, blkg32] fp16 (blkg = 8blk+g
     innermost so the extraction DMAs below get 576B contiguous runs).
  3. u-extract (x-shift): 16 DMAs on Sync (one per u, partitions u::16):
     t2x[m | dy', dx, blkg] = W[m | dy', u+dx, blkg]; src/dst runs 576B+.
  4. r-unshift (y-shift): 8 DMAs on GpSimd (one per r, partitions
     16r..16r+16): t2f[m | dy, dx, blkg] = t2x[m | r+dy, dx, blkg]; one
     contiguous 5.2KB run per partition.
  5. DVE reorder t2f -> t3[m | blkg, 81] so the PE transpose stationary
     has a single contiguous free dim.
  6. PE transpose per block-group: tt[k=81, m] <- t3[:, bg, :].
  7. ACT/DVE drain with 1/C scale to fp16 staging, one contiguous store
     per chunk; host upcasts to fp32.

v3 vs v2: u-first extraction (1.18+0.66 MB/chunk vs 1.77+0.66), Scalar
does no DMAs (was starving the psum drains), all stationary loads are
prefetched upfront, in2p load slices match per-chunk consumption so
chunk-0 matmuls start ~6us in, PSUM pools 3x2-bank mm tiles + 2x1-bank
transpose tiles.
"""

import numpy as np

import concourse.bass as bass
import concourse.mybir as mybir
from concourse import bacc
from concourse.bass_utils import run_bass_kernel_spmd
from concourse.masks import make_identity
from concourse.tile import TileContext

B = 8
C = 128
H = 96
W = 128
D = 9  # 2*max_disp + 1
K = D * D  # 81 output channels
PAD = 4
WP = W + 2 * PAD  # 136
FP32 = mybir.dt.float32
FP16 = mybir.dt.float16

N_CORES = 8
RCH = 32  # rows per chunk
BR = 8  # block rows
BU = 16  # block cols
NBLK = RCH // BR  # 4 row-blocks per chunk
NG = W // BU  # 8 col-groups
NBG = NBLK * NG  # 32 pixel-blocks per chunk
DYP = BR + 2 * PAD  # 16 dy' values per block
VP = BU + 2 * PAD  # 24 v values per group
NMM = DYP * VP  # 384 matmul free size
COPY = mybir.ActivationFunctionType.Copy


def build_bass(h: int = H):
    """Build the per-core Bass program for a [C, h, W] sample."""
    hp = h + 2 * PAD
    nch = h // RCH
    assert h % RCH == 0
    nc = bacc.Bacc(None, target_bir_lowering=False)
    # in1s is host-shuffled to [C, blk_total, g, m] with m = 16r+u,
    # y = 8*blk + r, x = 16*g + u (stationary needs one free dim).
    in1s = nc.dram_tensor("in1s", [C, h // BR, NG, 128], FP16, kind="ExternalInput")
    # in2p is host-padded: [C, h+8, W+8] with zeros in the 4-wide borders.
    in2p = nc.dram_tensor("in2p", [C, hp, WP], FP16, kind="ExternalInput")
    out = nc.dram_tensor("out", [K, h, W], FP16, kind="ExternalOutput")

    with TileContext(nc) as tc:
        with (
            tc.tile_pool(name="cst", bufs=1) as cst,
            tc.tile_pool(name="s1p", bufs=3) as s1p,
            tc.tile_pool(name="wp", bufs=2) as wp,
            tc.tile_pool(name="t2xp", bufs=2) as t2xp,
            tc.tile_pool(name="t2p", bufs=2) as t2p,
            tc.tile_pool(name="t3p", bufs=2) as t3p,
            tc.tile_pool(name="top", bufs=2) as top,
            tc.tile_pool(name="gpp", bufs=3, space="PSUM") as gpp,
            tc.tile_pool(name="ttp", bufs=2, space="PSUM") as ttp,
        ):
            s2p = cst.tile([C, hp, WP], FP16, name="s2p")
            ident = cst.tile([128, 128], FP16, name="ident")

            # Interleave input loads so chunk-0 compute starts earliest:
            # s1c0, in2p rows for blocks 0-1, rows for blocks 2-3, s1c1, ...
            s1cs = []
            row_hi = 0

            def _load_s1c(ch):
                s1c = s1p.tile(
                    [C, NBLK, NG, 128], FP16, name=f"s1c{ch}", tag=f"s1c{ch}"
                )
                # chunk 0 on Sync so its transfer overlaps the first in2p
                # slice on the GpSimd ring
                eng = nc.sync if ch == 0 else nc.gpsimd
                eng.dma_start(
                    s1c[:, :, :, :], in1s[:, ch * NBLK : (ch + 1) * NBLK, :, :]
                )
                s1cs.append(s1c)

            def _load_s2p(need):
                nonlocal row_hi
                need = min(need, hp)
                if need > row_hi:
                    nc.gpsimd.dma_start(
                        s2p[:, row_hi:need, :], in2p[:, row_hi:need, :]
                    )
                    row_hi = need

            _load_s1c(0)
            _load_s2p(2 * BR + 2 * PAD)  # rows for chunk-0 blocks 0-1
            _load_s2p(RCH + 2 * PAD)  # rest of chunk 0
            for ch in range(1, nch):
                _load_s1c(ch)
                _load_s2p(RCH * (ch + 1) + 2 * PAD if ch < nch - 1 else hp)

            make_identity(nc, ident)

            def head(ch):
                """MMs + psum drains + extraction DMAs for chunk ch."""
                y0 = ch * RCH
                s1c = s1cs[ch]

                # W[m | dy', v, blkg] fp16, blkg = 8*blk + g innermost
                wt = wp.tile([128, DYP, VP, NBG], FP16, name="wt", tag="wt")
                for blk in range(NBLK):
                    yb = y0 + BR * blk  # top row of block, in padded coords
                    for half in range(NG // 2):
                        gp = gpp.tile([128, 2, 512], FP32, name="gp", tag="gp")
                        for j in range(2):
                            g = 2 * half + j
                            nc.tensor.matmul(
                                gp[:, j, 0:NMM].rearrange(
                                    "p (dy v) -> p dy v", dy=DYP
                                ),
                                s1c[:, blk, g, :],
                                s2p[:, yb : yb + DYP, BU * g : BU * g + VP],
                                start=True,
                                stop=True,
                            )
                        # PSUM -> W (fp32 -> fp16); innermost j-pair contiguous
                        bg = NG * blk + 2 * half
                        dst = wt[:, :, :, bg : bg + 2]
                        src = gp[:, :, 0:NMM].rearrange(
                            "p j (dy v) -> p dy v j", dy=DYP
                        )
                        i = blk * 4 + half
                        if i % 2 == 1:
                            nc.vector.tensor_copy(dst, src)
                        else:
                            nc.scalar.activation(dst, src, COPY)

                # --- u-extract (x-shift): 16 DMAs on Sync, 576B runs ---
                # t2x[m | dy', dx, blkg] = wt[m | dy', u+dx, blkg], u = m%16
                t2x = t2xp.tile([128, DYP, D, NBG], FP16, name="t2x", tag="t2x")
                for u in range(BU):
                    nc.sync.dma_start(
                        t2x[u::BU, :, :, :],
                        wt[u::BU, :, u : u + D, :],
                    )

                # --- r-unshift (y-shift): 8 DMAs, 5.2KB runs ---
                # t2f[16r+u | dy, dx, blkg] = t2x[16r+u | r+dy, dx, blkg]
                # Last chunk: Sync (it is idle by then and skips the SWDGE
                # queue-drain stall on the exposed final tail).
                t2f = t2p.tile([128, D, D, NBG], FP16, name="t2f", tag="t2f")
                for r in range(BR):
                    if ch == nch - 1:
                        eng2 = nc.sync if r % 2 == 0 else nc.scalar
                    else:
                        eng2 = nc.gpsimd
                    eng2.dma_start(
                        t2f[BU * r : BU * r + BU, :, :, :],
                        t2x[BU * r : BU * r + BU, r : r + D, :, :],
                    )
                return t2f

            def tail(ch, t2f):
                """Reorder + PE transpose + scaled drain + store for chunk ch."""
                y0 = ch * RCH
                last = ch == nch - 1
                t3 = t3p.tile([128, NBG, D, D], FP16, name="t3", tag="t3")
                if last:
                    # split across both engines: latency is exposed here
                    # (uneven: ACT is ~2x slower on this permute than DVE)
                    sp = 20
                    nc.vector.tensor_copy(
                        t3[:, 0:sp, :, :],
                        t2f[:, :, :, 0:sp].rearrange("p dy dx b -> p b dy dx"),
                    )
                    nc.scalar.activation(
                        t3[:, sp:, :, :],
                        t2f[:, :, :, sp:].rearrange("p dy dx b -> p b dy dx"),
                        COPY,
                    )
                else:
                    nc.vector.tensor_copy(
                        t3[:, :, :, :],
                        t2f[:, :, :, :].rearrange("p dy dx b -> p b dy dx"),
                    )

                to = top.tile([K, RCH, W], FP16, name="to", tag="to")
                for blk in range(NBLK):
                    for hf in range(2):
                        tt = ttp.tile([K, 4, 128], FP16, name="tt", tag="tt")
                        for gi in range(4):
                            g = 4 * hf + gi
                            bg = NG * blk + g
                            nc.tensor.transpose(
                                tt[:, gi, :],
                                t3[:, bg, :, :].rearrange("p dy dx -> p (dy dx)"),
                                ident[:, :],
                            )
                        # to[k, 8blk+r, 16g+u] <- tt[k, gi, 16r+u]
                        dst = to[
                            :, BR * blk : BR * blk + BR, 64 * hf : 64 * hf + 64
                        ].rearrange("k r (g u) -> k r g u", g=4)
                        src = tt[:, :, :].rearrange("k g (r u) -> k r g u", r=BR)
                        if (blk * 2 + hf) % 2 == 0:
                            nc.scalar.activation(dst, src, COPY, scale=1.0 / C)
                        else:
                            nc.vector.tensor_scalar_mul(dst, src, 1.0 / C)
                    if last and blk == NBLK // 2 - 1:
                        # pipeline the exposed final store with the drains
                        nc.sync.dma_start(
                            out[:, y0 : y0 + RCH // 2, :],
                            to[:, 0 : RCH // 2, :],
                        )

                # --- store: contiguous fp16 block ---
                if last:
                    nc.sync.dma_start(
                        out[:, y0 + RCH // 2 : y0 + RCH, :],
                        to[:, RCH // 2 :, :],
                    )
                else:
                    nc.sync.dma_start(out[:, y0 : y0 + RCH, :], to[:, :, :])

            # Software-pipelined emission: chunk ch's tail is emitted after
            # chunk ch+1's head so each engine's static schedule interleaves
            # the extraction chain of one chunk with the compute of the next.
            prev = None
            for ch in range(nch):
                t2f = head(ch)
                if prev is not None:
                    tail(ch - 1, prev)
                prev = t2f
            tail(nch - 1, prev)

    nc.compile()
    return nc


_cached = {}


def _get_nc(h: int):
    if h not in _cached:
        _cached[h] = build_bass(h)
    return _cached[h]


def _pad_in2(in2: np.ndarray) -> np.ndarray:
    # [C, h, W] fp16 -> [C, h+8, W+8] zero-padded, contiguous fp16
    return np.pad(
        in2.astype(np.float16), ((0, 0), (PAD, PAD), (PAD, PAD)), mode="constant"
    )


def _shuffle_in1(in1: np.ndarray) -> np.ndarray:
    # [C, h, W] -> [C, h//8, 8(g), 128(m)] with m = 16r+u,
    # y = 8*blk + r, x = 16*g + u.
    c, h, w = in1.shape
    a = in1.astype(np.float16).reshape(c, h // BR, BR, NG, BU)  # c,blk,r,g,u
    a = a.transpose(0, 1, 3, 2, 4)  # c, blk, g, r, u
    return np.ascontiguousarray(a.reshape(c, h // BR, NG, 128))


def kernel(**inputs: np.ndarray) -> np.ndarray:
    in1 = np.asarray(inputs["in1"], dtype=np.float32)
    in2 = np.asarray(inputs["in2"], dtype=np.float32)
    assert in1.shape == (B, C, H, W), in1.shape

    nc = _get_nc(H)
    in_maps = [
        {
            "in1s": _shuffle_in1(in1[b]),
            "in2p": np.ascontiguousarray(_pad_in2(in2[b])),
        }
        for b in range(B)
    ]
    res = run_bass_kernel_spmd(nc, in_maps, core_ids=list(range(N_CORES)))
    return np.stack([r["out"] for r in res.results], axis=0).astype(np.float32)
